# revision 13
# baseline (speedup 1.0000x reference)
"""Trainium2 Bass kernel for nn_MultiHeadAttention (B=2, N=4096, D=512, H=8).

Sharding: 8 cores = 2 batch groups x 4 head-pair shards.
Core c: batch b = c//4, head pair hp = c%4 (global heads 2hp, 2hp+1),
ReduceScatter rank = c%4 within its batch group.

Per-core device pipeline:
  - projections: qpT/kpT [128(2 heads x 64), 4096] fp16, vp [4096, 128] fp16
  - stage B in 8 groups of 4 row-chunks (128 rows each), heads inner:
      scores[n,m] = kp[n]·qp[m] (fp16 matmuls, f32 psum, softmax scale
      pre-folded into w_q), exp on ScalarE with accum_out denominators,
      in-place per-partition normalize (VectorE), attn row-block written
      to DRAM via SWDGE cast-DMA fp16->f32, PE-transposed into 512-wide
      rhs tiles for the feat matmuls (featT accumulated in PSUM).
  - per group: w_fc projection partial; per 2 groups: chunked
    ReduceScatter(add) over the 4-core batch group, then +q residual,
    LayerNorm, write 256 rows of `out` (pipelined with attention).
Returns (out [2,4096,512], attn [2,8,4096,4096]) matching the reference.
"""

import os
import sys
import types

import numpy as np

# NTFF profile hook shim: this image's antenv lacks axon_hooks; bass_utils
# imports it unconditionally when trace=True under axon.
try:
    from trn_agent_boot.trn_boot import _ntff_profile_via_ctypes

    _hook = _ntff_profile_via_ctypes("/opt/axon/libaxon_pjrt.so")
    _mod = types.ModuleType("antenv.axon_hooks")
    _mod.get_axon_ntff_profile_hook = lambda: _hook
    _mod.set_axon_ntff_profile_hook = lambda h: None
    sys.modules.setdefault("antenv.axon_hooks", _mod)
except Exception:
    pass

import concourse.bacc as bacc
import concourse.bass as bass
import concourse.mybir as mybir
import concourse.tile as tile
from concourse.bass_utils import run_bass_kernel_spmd
from concourse.masks import make_identity

F32 = mybir.dt.float32
F16 = mybir.dt.float16

B, N, D, H = 2, 4096, 512, 8
DK = D // H  # 64
NCORES = 8
GROUPS = [[0, 1, 2, 3], [4, 5, 6, 7]]
NSLICE = N // 4  # rows of `out` per core (4 bands of 256)
BAND = NSLICE // 4  # 256
EPS = 1e-5

P = 128
NCHUNKS = N // P      # 32 row chunks
NGROUPS = NCHUNKS // 4  # 8 groups of 4 chunks
MB = 1024             # m-block width for scores psum / exp calls

last_exec_time_ns = None
_cached = None


def _build():
    nc = bacc.Bacc(None, target_bir_lowering=False)

    qT = nc.dram_tensor("qT", [D, N], F32, kind="ExternalInput")
    kT = nc.dram_tensor("kT", [D, N], F32, kind="ExternalInput")
    vT = nc.dram_tensor("vT", [D, N], F32, kind="ExternalInput")
    q_res = nc.dram_tensor("q_res", [NSLICE, D], F32, kind="ExternalInput")
    wqT = nc.dram_tensor("wqT", [D, P], F32, kind="ExternalInput")
    wkT = nc.dram_tensor("wkT", [D, P], F32, kind="ExternalInput")
    wvT = nc.dram_tensor("wvT", [D, P], F32, kind="ExternalInput")
    wfcT = nc.dram_tensor("wfcT", [P, D], F32, kind="ExternalInput")
    gamma = nc.dram_tensor("gamma", [D], F32, kind="ExternalInput")
    beta = nc.dram_tensor("beta", [D], F32, kind="ExternalInput")

    attn_out = nc.dram_tensor("attn_part", [2, N, N], F32, kind="ExternalOutput")
    out_part = nc.dram_tensor("out_part", [NSLICE, D], F32, kind="ExternalOutput")

    def bcast_row(dram_ap, parts):
        return bass.AP(
            tensor=dram_ap.tensor,
            offset=dram_ap.offset,
            ap=[[0, parts]] + list(dram_ap.ap),
        )

    with tile.TileContext(nc) as tc:
        with (
            tc.tile_pool(name="const", bufs=1) as const,
            tc.tile_pool(name="persist", bufs=1) as persist,
            tc.tile_pool(name="xt", bufs=4) as xt_pool,
            tc.tile_pool(name="ework", bufs=12) as ework,
            tc.tile_pool(name="etw", bufs=6) as etw,
            tc.tile_pool(name="small", bufs=3) as small,
            tc.tile_pool(name="cstage", bufs=3) as cstage,
            tc.tile_pool(name="sc", bufs=2, space="PSUM") as sc_pool,
            tc.tile_pool(name="pt", bufs=3, space="PSUM") as pt_pool,
            tc.tile_pool(name="sm", bufs=1, space="PSUM") as sm_pool,
            tc.tile_pool(name="dram", bufs=1, space="DRAM") as dram,
        ):
            # ---- constants ----
            ident = const.tile([P, P], F16)
            make_identity(nc, ident)
            wq_sb = const.tile([P, 4, P], F16)
            wk_sb = const.tile([P, 4, P], F16)
            wv_sb = const.tile([P, 4, P], F16)
            for w_dram, w_sb in ((wqT, wq_sb), (wkT, wk_sb), (wvT, wv_sb)):
                for dc in range(4):
                    nc.gpsimd.dma_start(
                        out=w_sb[:, dc, :], in_=w_dram[dc * P : (dc + 1) * P, :]
                    )
            wfc_f32 = const.tile([P, D], F32)
            nc.sync.dma_start(out=wfc_f32[:], in_=wfcT[:])
            wfc_16 = const.tile([P, D], F16)
            nc.scalar.activation(
                out=wfc_16[:], in_=wfc_f32[:], func=mybir.ActivationFunctionType.Copy
            )
            gamma_bc = const.tile([P, D], F32)
            beta_bc = const.tile([P, D], F32)
            nc.gpsimd.dma_start(out=gamma_bc[:], in_=bcast_row(gamma[:], P))
            nc.gpsimd.dma_start(out=beta_bc[:], in_=bcast_row(beta[:], P))
            eps_sb = const.tile([P, 1], F32)
            nc.vector.memset(eps_sb[:], EPS)

            # ---- persistent projections ----
            qpT = persist.tile([P, N], F16)
            kpT = persist.tile([P, N], F16)
            vp = persist.tile([P, NCHUNKS, P], F16)
            featT = persist.tile([P, N], F16)

            # ---- stage A: q/k projections first (attention depends on them),
            # v projection after (only needed by the first feat phase) ----
            for x_dram, w_sb, out_sb in ((qT, wq_sb, qpT), (kT, wk_sb, kpT)):
                x_tiles = [
                    xt_pool.tile([P, N], F16, name=f"xt{dc}", tag="xt")
                    for dc in range(4)
                ]
                for dc in range(4):
                    nc.gpsimd.dma_start(
                        out=x_tiles[dc][:], in_=x_dram[dc * P : (dc + 1) * P, :]
                    )
                for nq in range(4):
                    pp = sc_pool.tile([P, MB], F32, tag="sc")
                    for dc in range(4):
                        for half in range(2):
                            nc.tensor.matmul(
                                pp[:, half * 512 : (half + 1) * 512],
                                w_sb[:, dc, :],
                                x_tiles[dc][
                                    :,
                                    nq * MB + half * 512 : nq * MB + (half + 1) * 512,
                                ],
                                start=(dc == 0),
                                stop=(dc == 3),
                            )
                    nc.scalar.activation(
                        out=out_sb[:, nq * MB : (nq + 1) * MB],
                        in_=pp[:],
                        func=mybir.ActivationFunctionType.Copy,
                    )

            vt_tiles = [
                xt_pool.tile([P, N], F16, name=f"vt{dc}", tag="xt") for dc in range(4)
            ]
            for dc in range(4):
                nc.gpsimd.dma_start(
                    out=vt_tiles[dc][:], in_=vT[dc * P : (dc + 1) * P, :]
                )
            for mc in range(NCHUNKS):
                vp_ps = sm_pool.tile([P, P], F32, tag="sm")
                for dc in range(4):
                    nc.tensor.matmul(
                        vp_ps[:],
                        vt_tiles[dc][:, mc * P : (mc + 1) * P],
                        wv_sb[:, dc, :],
                        start=(dc == 0),
                        stop=(dc == 3),
                    )
                nc.scalar.activation(
                    out=vp[:, mc, :],
                    in_=vp_ps[:],
                    func=mybir.ActivationFunctionType.Copy,
                )

            # ---- stage B + pipelined stage C ----
            fproj = dram.tile([N, D], F32)
            rs_chunks = [
                dram.tile([BAND, D], F32, name=f"rs{i}") for i in range(4)
            ]

            def emit_scores(g, h, a_norms):
                qpT_h = qpT[h * DK : (h + 1) * DK, :]
                kpT_h = kpT[h * DK : (h + 1) * DK, :]
                if True:
                    for j in range(4):
                        ci = g * 4 + j
                        n0 = ci * P
                        ea = ework.tile([P, N], F16, name=f"ea{j}", tag="ea")
                        den4 = small.tile([P, 4], F32, tag="den4")
                        for mb in range(N // MB):
                            sp = sc_pool.tile([P, MB], F32, tag="sc")
                            for half in range(2):
                                nc.tensor.matmul(
                                    sp[:, half * 512 : (half + 1) * 512],
                                    kpT_h[:, n0 : n0 + P],
                                    qpT_h[
                                        :,
                                        mb * MB + half * 512 : mb * MB
                                        + (half + 1) * 512,
                                    ],
                                    start=True,
                                    stop=True,
                                )
                            nc.scalar.activation(
                                out=ea[:, mb * MB : (mb + 1) * MB],
                                in_=sp[:],
                                func=mybir.ActivationFunctionType.Exp,
                                accum_out=den4[:, mb : mb + 1],
                            )
                            yield
                        den = small.tile([P, 1], F32, tag="den")
                        nc.vector.tensor_reduce(
                            out=den[:],
                            in_=den4[:],
                            axis=mybir.AxisListType.X,
                            op=mybir.AluOpType.add,
                        )
                        rden = small.tile([P, 1], F32, tag="rden")
                        nc.vector.reciprocal(out=rden[:], in_=den[:])
                        # in-place normalize -> fp16 attn row-block
                        nc.vector.tensor_scalar_mul(
                            out=ea[:], in0=ea[:], scalar1=rden[:]
                        )
                        nc.gpsimd.dma_start(
                            out=attn_out[h, n0 : n0 + P, :], in_=ea[:]
                        )
                        a_norms.append(ea)

            def emit_feat(g, h, a_norms):
                fp = sm_pool.tile([DK, 512], F32, tag="sm", name="fp")
                for slab in range(NCHUNKS):
                    yield
                    ptile = pt_pool.tile([P, 512], F16, tag="pt", name="ptile")
                    for j in range(4):
                        nc.tensor.transpose(
                            ptile[:, j * P : (j + 1) * P],
                            a_norms[j][:, slab * P : (slab + 1) * P],
                            ident[:],
                        )
                    et = etw.tile([P, 512], F16, tag="et", name="et")
                    nc.vector.tensor_copy(out=et[:], in_=ptile[:])
                    nc.tensor.matmul(
                        fp[:],
                        vp[:, slab, h * DK : (h + 1) * DK],
                        et[:],
                        start=(slab == 0),
                        stop=(slab == NCHUNKS - 1),
                    )
                nc.scalar.activation(
                    out=featT[h * DK : (h + 1) * DK, g * 512 : (g + 1) * 512],
                    in_=fp[:],
                    func=mybir.ActivationFunctionType.Copy,
                )

            def emit_proj(g):
                # w_fc projection for this group's 512 columns (both heads done)
                for t in range(4):
                    ci = g * 4 + t
                    pp = sc_pool.tile([P, D], F32, tag="sc", name="pp")
                    nc.tensor.matmul(
                        pp[:],
                        featT[:, ci * P : (ci + 1) * P],
                        wfc_16[:],
                        start=True,
                        stop=True,
                    )
                    st = cstage.tile([P, D], F32, tag="cst")
                    nc.scalar.activation(
                        out=st[:], in_=pp[:], func=mybir.ActivationFunctionType.Copy
                    )
                    nc.sync.dma_start(out=fproj[ci * P : (ci + 1) * P, :], in_=st[:])

            def emit_rs(qrt):
                if True:
                    nc.gpsimd.collective_compute(
                        "ReduceScatter",
                        mybir.AluOpType.add,
                        replica_groups=GROUPS,
                        ins=[fproj[qrt * 1024 : (qrt + 1) * 1024, :].opt()],
                        outs=[rs_chunks[qrt][:].opt()],
                    )
                    for ri in range(BAND // P):
                        row = qrt * BAND + ri * P
                        xt = cstage.tile([P, D], F32, tag="cx")
                        nc.sync.dma_start(
                            out=xt[:], in_=rs_chunks[qrt][ri * P : (ri + 1) * P, :]
                        )
                        qt = cstage.tile([P, D], F32, tag="cq")
                        nc.sync.dma_start(out=qt[:], in_=q_res[row : row + P, :])
                        nc.vector.tensor_add(out=xt[:], in0=xt[:], in1=qt[:])
                        stats = small.tile([P, 6], F32, tag="bnst")
                        nc.vector.bn_stats(out=stats[:], in_=xt[:])
                        mv = small.tile([P, 2], F32, tag="bnmv")
                        nc.vector.bn_aggr(out=mv[:], in_=stats[:])
                        rstd = small.tile([P, 1], F32, tag="rstd")
                        nc.scalar.activation(
                            out=rstd[:],
                            in_=mv[:, 1:2],
                            func=mybir.ActivationFunctionType.Sqrt,
                            bias=eps_sb[:],
                        )
                        nc.vector.reciprocal(out=rstd[:], in_=rstd[:])
                        nc.vector.tensor_scalar(
                            out=xt[:],
                            in0=xt[:],
                            scalar1=mv[:, 0:1],
                            scalar2=rstd[:],
                            op0=mybir.AluOpType.subtract,
                            op1=mybir.AluOpType.mult,
                        )
                        nc.vector.tensor_mul(out=xt[:], in0=xt[:], in1=gamma_bc[:])
                        nc.vector.tensor_add(out=xt[:], in0=xt[:], in1=beta_bc[:])
                        nc.sync.dma_start(out=out_part[row : row + P, :], in_=xt[:])

            # software pipeline, fine-grained: interleave score units of
            # step i+1 with transpose/feat units of step i so the PE FIFO
            # alternates between them (avoids head-of-line blocking while
            # scores wait on exp's psum slots).
            def drain(it, k):
                for _ in range(k):
                    if next(it, "end") == "end":
                        return False
                return True

            pending = None  # (g, h, feat generator)
            steps = [(g, h) for g in range(NGROUPS) for h in range(2)]
            for g, h in steps:
                a_norms = []
                sgen = emit_scores(g, h, a_norms)
                fgen = None if pending is None else pending[2]
                while True:
                    alive = drain(sgen, 1)
                    if fgen is not None:
                        drain(fgen, 2)
                    if not alive:
                        break
                if pending is not None:
                    pg, ph, pf = pending
                    drain(pf, 64)  # finish any remainder
                    if ph == 1:
                        emit_proj(pg)
                        if pg % 2 == 1:
                            emit_rs(pg // 2)
                pending = (g, h, emit_feat(g, h, a_norms))
            pg, ph, pf = pending
            drain(pf, 64)
            emit_proj(pg)
            emit_rs(pg // 2)

    if not nc.is_finalized():
        nc.finalize()
    return nc


def kernel(q, k, v, w_q, w_k, w_v, w_fc, ln_gamma, ln_beta):
    global last_exec_time_ns, _cached
    q = np.asarray(q, dtype=np.float32)
    k = np.asarray(k, dtype=np.float32)
    v = np.asarray(v, dtype=np.float32)
    w_q = np.asarray(w_q, dtype=np.float32)
    w_k = np.asarray(w_k, dtype=np.float32)
    w_v = np.asarray(w_v, dtype=np.float32)
    w_fc = np.asarray(w_fc, dtype=np.float32)
    ln_gamma = np.asarray(ln_gamma, dtype=np.float32)
    ln_beta = np.asarray(ln_beta, dtype=np.float32)

    if _cached is None:
        _cached = _build()
    nc = _cached

    scale = np.float32(1.0 / np.sqrt(np.float32(DK)))
    in_maps = []
    for c in range(NCORES):
        b, hp = c // 4, c % 4
        rank = c % 4
        # out rows this core produces: band q covers q*1024 + rank*256
        rows = np.concatenate(
            [
                np.arange(nq * 1024 + rank * BAND, nq * 1024 + (rank + 1) * BAND)
                for nq in range(4)
            ]
        )
        in_maps.append(
            {
                "qT": np.ascontiguousarray(q[b].T),
                "kT": np.ascontiguousarray(k[b].T),
                "vT": np.ascontiguousarray(v[b].T),
                "q_res": np.ascontiguousarray(q[b][rows]),
                "wqT": np.ascontiguousarray((w_q[P * hp : P * (hp + 1), :] * scale).T),
                "wkT": np.ascontiguousarray(w_k[P * hp : P * (hp + 1), :].T),
                "wvT": np.ascontiguousarray(w_v[P * hp : P * (hp + 1), :].T),
                "wfcT": np.ascontiguousarray(w_fc[:, P * hp : P * (hp + 1)].T),
                "gamma": ln_gamma,
                "beta": ln_beta,
            }
        )

    trace = os.environ.get("ATTN_TRACE", "0") == "1"
    res = run_bass_kernel_spmd(nc, in_maps, list(range(NCORES)), trace=trace)
    last_exec_time_ns = res.exec_time_ns

    attn = np.empty((B, H, N, N), dtype=np.float32)
    out = np.empty((B, N, D), dtype=np.float32)
    for c in range(NCORES):
        b, hp = c // 4, c % 4
        rank = c % 4
        attn[b, 2 * hp : 2 * hp + 2] = res.results[c]["attn_part"]
        op = res.results[c]["out_part"]
        for nq in range(4):
            out[b, nq * 1024 + rank * BAND : nq * 1024 + (rank + 1) * BAND] = op[
                nq * BAND : (nq + 1) * BAND
            ]
    return out, attn


# revision 16
# speedup vs baseline: 1.1633x; 1.1633x over previous
"""Trainium2 Bass kernel for nn_MultiHeadAttention (B=2, N=4096, D=512, H=8).

Sharding: 8 cores = 2 batch groups x 4 head-pair shards.
Core c: batch b = c//4, head pair hp = c%4 (global heads 2hp, 2hp+1),
ReduceScatter rank = c%4 within its batch group.

Per-core device pipeline:
  - projections: qpT/kpT [128(2 heads x 64), 4096] fp16, vp [4096, 128] fp16
  - stage B in 8 groups of 4 row-chunks (128 rows each), heads inner:
      scores[n,m] = kp[n]·qp[m] (fp16 matmuls, f32 psum, softmax scale
      pre-folded into w_q), exp on ScalarE with accum_out denominators,
      in-place per-partition normalize (VectorE), attn row-block written
      to DRAM via SWDGE cast-DMA fp16->f32, PE-transposed into 512-wide
      rhs tiles for the feat matmuls (featT accumulated in PSUM).
  - per group: w_fc projection partial; per 2 groups: chunked
    ReduceScatter(add) over the 4-core batch group, then +q residual,
    LayerNorm, write 256 rows of `out` (pipelined with attention).
Returns (out [2,4096,512], attn [2,8,4096,4096]) matching the reference.
"""

import os
import sys
import types

import numpy as np

# NTFF profile hook shim: this image's antenv lacks axon_hooks; bass_utils
# imports it unconditionally when trace=True under axon.
try:
    from trn_agent_boot.trn_boot import _ntff_profile_via_ctypes

    _hook = _ntff_profile_via_ctypes("/opt/axon/libaxon_pjrt.so")
    _mod = types.ModuleType("antenv.axon_hooks")
    _mod.get_axon_ntff_profile_hook = lambda: _hook
    _mod.set_axon_ntff_profile_hook = lambda h: None
    sys.modules.setdefault("antenv.axon_hooks", _mod)
except Exception:
    pass

import concourse.bacc as bacc
import concourse.bass as bass
import concourse.mybir as mybir
import concourse.tile as tile
from concourse.bass_utils import run_bass_kernel_spmd
from concourse.masks import make_identity

F32 = mybir.dt.float32
F16 = mybir.dt.float16

B, N, D, H = 2, 4096, 512, 8
DK = D // H  # 64
NCORES = 8
GROUPS = [[0, 1, 2, 3], [4, 5, 6, 7]]
NSLICE = N // 4  # rows of `out` per core (4 bands of 256)
BAND = NSLICE // 4  # 256
EPS = 1e-5

P = 128
NCHUNKS = N // P      # 32 row chunks
NGROUPS = NCHUNKS // 4  # 8 groups of 4 chunks
MB = 1024             # m-block width for scores psum / exp calls

last_exec_time_ns = None
_cached = None


def _build():
    nc = bacc.Bacc(None, target_bir_lowering=False)

    qT = nc.dram_tensor("qT", [D, N], F32, kind="ExternalInput")
    kT = nc.dram_tensor("kT", [D, N], F32, kind="ExternalInput")
    vT = nc.dram_tensor("vT", [D, N], F32, kind="ExternalInput")
    q_res = nc.dram_tensor("q_res", [NSLICE, D], F32, kind="ExternalInput")
    wqT = nc.dram_tensor("wqT", [D, P], F32, kind="ExternalInput")
    wkT = nc.dram_tensor("wkT", [D, P], F32, kind="ExternalInput")
    wvT = nc.dram_tensor("wvT", [D, P], F32, kind="ExternalInput")
    wfcT = nc.dram_tensor("wfcT", [P, D], F32, kind="ExternalInput")
    gamma = nc.dram_tensor("gamma", [D], F32, kind="ExternalInput")
    beta = nc.dram_tensor("beta", [D], F32, kind="ExternalInput")

    attn_out = nc.dram_tensor("attn_part", [2, N, N], F32, kind="ExternalOutput")
    out_part = nc.dram_tensor("out_part", [NSLICE, D], F32, kind="ExternalOutput")

    def bcast_row(dram_ap, parts):
        return bass.AP(
            tensor=dram_ap.tensor,
            offset=dram_ap.offset,
            ap=[[0, parts]] + list(dram_ap.ap),
        )

    with tile.TileContext(nc) as tc:
        with (
            tc.tile_pool(name="const", bufs=1) as const,
            tc.tile_pool(name="persist", bufs=1) as persist,
            tc.tile_pool(name="xt", bufs=4) as xt_pool,
            tc.tile_pool(name="ework", bufs=12) as ework,
            tc.tile_pool(name="etw", bufs=6) as etw,
            tc.tile_pool(name="small", bufs=3) as small,
            tc.tile_pool(name="cstage", bufs=3) as cstage,
            tc.tile_pool(name="sc", bufs=2, space="PSUM") as sc_pool,
            tc.tile_pool(name="pt", bufs=3, space="PSUM") as pt_pool,
            tc.tile_pool(name="sm", bufs=1, space="PSUM") as sm_pool,
            tc.tile_pool(name="dram", bufs=1, space="DRAM") as dram,
        ):
            # ---- constants ----
            ident = const.tile([P, P], F16)
            make_identity(nc, ident)
            wq_sb = const.tile([P, 4, P], F16)
            wk_sb = const.tile([P, 4, P], F16)
            wv_sb = const.tile([P, 4, P], F16)
            for w_dram, w_sb in ((wqT, wq_sb), (wkT, wk_sb), (wvT, wv_sb)):
                for dc in range(4):
                    nc.gpsimd.dma_start(
                        out=w_sb[:, dc, :], in_=w_dram[dc * P : (dc + 1) * P, :]
                    )
            wfc_f32 = const.tile([P, D], F32)
            nc.sync.dma_start(out=wfc_f32[:], in_=wfcT[:])
            wfc_16 = const.tile([P, D], F16)
            nc.scalar.activation(
                out=wfc_16[:], in_=wfc_f32[:], func=mybir.ActivationFunctionType.Copy
            )
            gamma_bc = const.tile([P, D], F32)
            beta_bc = const.tile([P, D], F32)
            nc.gpsimd.dma_start(out=gamma_bc[:], in_=bcast_row(gamma[:], P))
            nc.gpsimd.dma_start(out=beta_bc[:], in_=bcast_row(beta[:], P))
            eps_sb = const.tile([P, 1], F32)
            nc.vector.memset(eps_sb[:], EPS)

            # ---- persistent projections ----
            qpT = persist.tile([P, N], F16)
            kpT = persist.tile([P, N], F16)
            vp = persist.tile([P, NCHUNKS, P], F16)
            featT = persist.tile([P, N], F16)

            # ---- stage A: q/k projections first (attention depends on them),
            # v projection after (only needed by the first feat phase) ----
            for x_dram, w_sb, out_sb in ((qT, wq_sb, qpT), (kT, wk_sb, kpT)):
                x_tiles = [
                    xt_pool.tile([P, N], F16, name=f"xt{dc}", tag="xt")
                    for dc in range(4)
                ]
                for dc in range(4):
                    nc.gpsimd.dma_start(
                        out=x_tiles[dc][:], in_=x_dram[dc * P : (dc + 1) * P, :]
                    )
                for nq in range(4):
                    pp = sc_pool.tile([P, MB], F32, tag="sc")
                    for dc in range(4):
                        for half in range(2):
                            nc.tensor.matmul(
                                pp[:, half * 512 : (half + 1) * 512],
                                w_sb[:, dc, :],
                                x_tiles[dc][
                                    :,
                                    nq * MB + half * 512 : nq * MB + (half + 1) * 512,
                                ],
                                start=(dc == 0),
                                stop=(dc == 3),
                            )
                    nc.scalar.activation(
                        out=out_sb[:, nq * MB : (nq + 1) * MB],
                        in_=pp[:],
                        func=mybir.ActivationFunctionType.Copy,
                    )

            vt_tiles = [
                xt_pool.tile([P, N], F16, name=f"vt{dc}", tag="xt") for dc in range(4)
            ]
            for dc in range(4):
                nc.gpsimd.dma_start(
                    out=vt_tiles[dc][:], in_=vT[dc * P : (dc + 1) * P, :]
                )
            for mc in range(NCHUNKS):
                vp_ps = sm_pool.tile([P, P], F32, tag="sm")
                for dc in range(4):
                    nc.tensor.matmul(
                        vp_ps[:],
                        vt_tiles[dc][:, mc * P : (mc + 1) * P],
                        wv_sb[:, dc, :],
                        start=(dc == 0),
                        stop=(dc == 3),
                    )
                nc.scalar.activation(
                    out=vp[:, mc, :],
                    in_=vp_ps[:],
                    func=mybir.ActivationFunctionType.Copy,
                )

            # ---- stage B + pipelined stage C ----
            fproj = dram.tile([N, D], F32)
            rs_chunks = [
                dram.tile([BAND, D], F32, name=f"rs{i}") for i in range(4)
            ]

            def emit_scores(g, h, a_norms):
                qpT_h = qpT[h * DK : (h + 1) * DK, :]
                kpT_h = kpT[h * DK : (h + 1) * DK, :]
                if True:
                    for j in range(4):
                        ci = g * 4 + j
                        n0 = ci * P
                        ea = ework.tile([P, N], F16, name=f"ea{j}", tag="ea")
                        den4 = small.tile([P, 4], F32, tag="den4")
                        for mb in range(N // MB):
                            sp = sc_pool.tile([P, MB], F32, tag="sc")
                            for half in range(2):
                                nc.tensor.matmul(
                                    sp[:, half * 512 : (half + 1) * 512],
                                    kpT_h[:, n0 : n0 + P],
                                    qpT_h[
                                        :,
                                        mb * MB + half * 512 : mb * MB
                                        + (half + 1) * 512,
                                    ],
                                    start=True,
                                    stop=True,
                                )
                            nc.scalar.activation(
                                out=ea[:, mb * MB : (mb + 1) * MB],
                                in_=sp[:],
                                func=mybir.ActivationFunctionType.Exp,
                                accum_out=den4[:, mb : mb + 1],
                            )
                            yield
                        den = small.tile([P, 1], F32, tag="den")
                        nc.vector.tensor_reduce(
                            out=den[:],
                            in_=den4[:],
                            axis=mybir.AxisListType.X,
                            op=mybir.AluOpType.add,
                        )
                        rden = small.tile([P, 1], F32, tag="rden")
                        nc.vector.reciprocal(out=rden[:], in_=den[:])
                        # in-place normalize -> fp16 attn row-block
                        nc.vector.tensor_scalar_mul(
                            out=ea[:], in0=ea[:], scalar1=rden[:]
                        )
                        nc.gpsimd.dma_start(
                            out=attn_out[h, n0 : n0 + P, :], in_=ea[:]
                        )
                        a_norms.append(ea)

            def emit_feat(g, h, a_norms):
                fp = sm_pool.tile([DK, 512], F32, tag="sm", name="fp")
                for slab in range(NCHUNKS):
                    yield
                    ptile = pt_pool.tile([P, 512], F16, tag="pt", name="ptile")
                    for j in range(4):
                        nc.tensor.transpose(
                            ptile[:, j * P : (j + 1) * P],
                            a_norms[j][:, slab * P : (slab + 1) * P],
                            ident[:],
                        )
                    et = etw.tile([P, 512], F16, tag="et", name="et")
                    nc.vector.tensor_copy(out=et[:], in_=ptile[:])
                    nc.tensor.matmul(
                        fp[:],
                        vp[:, slab, h * DK : (h + 1) * DK],
                        et[:],
                        start=(slab == 0),
                        stop=(slab == NCHUNKS - 1),
                    )
                nc.scalar.activation(
                    out=featT[h * DK : (h + 1) * DK, g * 512 : (g + 1) * 512],
                    in_=fp[:],
                    func=mybir.ActivationFunctionType.Copy,
                )

            def emit_proj(g):
                # w_fc projection for this group's 512 columns (both heads done)
                for t in range(4):
                    ci = g * 4 + t
                    pp = sm_pool.tile([P, D], F32, tag="sm", name="pp")
                    nc.tensor.matmul(
                        pp[:],
                        featT[:, ci * P : (ci + 1) * P],
                        wfc_16[:],
                        start=True,
                        stop=True,
                    )
                    st = cstage.tile([P, D], F32, tag="cst")
                    nc.scalar.activation(
                        out=st[:], in_=pp[:], func=mybir.ActivationFunctionType.Copy
                    )
                    nc.sync.dma_start(out=fproj[ci * P : (ci + 1) * P, :], in_=st[:])

            def emit_rs(qrt):
                if True:
                    nc.gpsimd.collective_compute(
                        "ReduceScatter",
                        mybir.AluOpType.add,
                        replica_groups=GROUPS,
                        ins=[fproj[qrt * 1024 : (qrt + 1) * 1024, :].opt()],
                        outs=[rs_chunks[qrt][:].opt()],
                    )
                    for ri in range(BAND // P):
                        row = qrt * BAND + ri * P
                        xt = cstage.tile([P, D], F32, tag="cx")
                        nc.sync.dma_start(
                            out=xt[:], in_=rs_chunks[qrt][ri * P : (ri + 1) * P, :]
                        )
                        qt = cstage.tile([P, D], F32, tag="cq")
                        nc.sync.dma_start(out=qt[:], in_=q_res[row : row + P, :])
                        nc.vector.tensor_add(out=xt[:], in0=xt[:], in1=qt[:])
                        stats = small.tile([P, 6], F32, tag="bnst")
                        nc.vector.bn_stats(out=stats[:], in_=xt[:])
                        mv = small.tile([P, 2], F32, tag="bnmv")
                        nc.vector.bn_aggr(out=mv[:], in_=stats[:])
                        rstd = small.tile([P, 1], F32, tag="rstd")
                        nc.scalar.activation(
                            out=rstd[:],
                            in_=mv[:, 1:2],
                            func=mybir.ActivationFunctionType.Sqrt,
                            bias=eps_sb[:],
                        )
                        nc.vector.reciprocal(out=rstd[:], in_=rstd[:])
                        nc.vector.tensor_scalar(
                            out=xt[:],
                            in0=xt[:],
                            scalar1=mv[:, 0:1],
                            scalar2=rstd[:],
                            op0=mybir.AluOpType.subtract,
                            op1=mybir.AluOpType.mult,
                        )
                        nc.vector.tensor_mul(out=xt[:], in0=xt[:], in1=gamma_bc[:])
                        nc.vector.tensor_add(out=xt[:], in0=xt[:], in1=beta_bc[:])
                        nc.sync.dma_start(out=out_part[row : row + P, :], in_=xt[:])

            # software pipeline, fine-grained: interleave score units of
            # step i+1 with transpose/feat units of step i so the PE FIFO
            # alternates between them (avoids head-of-line blocking while
            # scores wait on exp's psum slots).
            def drain(it, k):
                for _ in range(k):
                    if next(it, "end") == "end":
                        return False
                return True

            pending = None  # (g, h, feat generator)
            steps = [(g, h) for g in range(NGROUPS) for h in range(2)]
            for g, h in steps:
                a_norms = []
                sgen = emit_scores(g, h, a_norms)
                fgen = None if pending is None else pending[2]
                while True:
                    alive = drain(sgen, 1)
                    if fgen is not None:
                        drain(fgen, 2)
                    if not alive:
                        break
                if pending is not None:
                    pg, ph, pf = pending
                    drain(pf, 64)  # finish any remainder
                    if ph == 1:
                        emit_proj(pg)
                        if pg % 2 == 1:
                            emit_rs(pg // 2)
                pending = (g, h, emit_feat(g, h, a_norms))
            pg, ph, pf = pending
            drain(pf, 64)
            emit_proj(pg)
            emit_rs(pg // 2)

    if not nc.is_finalized():
        nc.finalize()
    return nc


def kernel(q, k, v, w_q, w_k, w_v, w_fc, ln_gamma, ln_beta):
    global last_exec_time_ns, _cached
    q = np.asarray(q, dtype=np.float32)
    k = np.asarray(k, dtype=np.float32)
    v = np.asarray(v, dtype=np.float32)
    w_q = np.asarray(w_q, dtype=np.float32)
    w_k = np.asarray(w_k, dtype=np.float32)
    w_v = np.asarray(w_v, dtype=np.float32)
    w_fc = np.asarray(w_fc, dtype=np.float32)
    ln_gamma = np.asarray(ln_gamma, dtype=np.float32)
    ln_beta = np.asarray(ln_beta, dtype=np.float32)

    if _cached is None:
        _cached = _build()
    nc = _cached

    scale = np.float32(1.0 / np.sqrt(np.float32(DK)))
    in_maps = []
    for c in range(NCORES):
        b, hp = c // 4, c % 4
        rank = c % 4
        # out rows this core produces: band q covers q*1024 + rank*256
        rows = np.concatenate(
            [
                np.arange(nq * 1024 + rank * BAND, nq * 1024 + (rank + 1) * BAND)
                for nq in range(4)
            ]
        )
        in_maps.append(
            {
                "qT": np.ascontiguousarray(q[b].T),
                "kT": np.ascontiguousarray(k[b].T),
                "vT": np.ascontiguousarray(v[b].T),
                "q_res": np.ascontiguousarray(q[b][rows]),
                "wqT": np.ascontiguousarray((w_q[P * hp : P * (hp + 1), :] * scale).T),
                "wkT": np.ascontiguousarray(w_k[P * hp : P * (hp + 1), :].T),
                "wvT": np.ascontiguousarray(w_v[P * hp : P * (hp + 1), :].T),
                "wfcT": np.ascontiguousarray(w_fc[:, P * hp : P * (hp + 1)].T),
                "gamma": ln_gamma,
                "beta": ln_beta,
            }
        )

    trace = os.environ.get("ATTN_TRACE", "0") == "1"
    res = run_bass_kernel_spmd(nc, in_maps, list(range(NCORES)), trace=trace)
    last_exec_time_ns = res.exec_time_ns

    attn = np.empty((B, H, N, N), dtype=np.float32)
    out = np.empty((B, N, D), dtype=np.float32)
    for c in range(NCORES):
        b, hp = c // 4, c % 4
        rank = c % 4
        attn[b, 2 * hp : 2 * hp + 2] = res.results[c]["attn_part"]
        op = res.results[c]["out_part"]
        for nq in range(4):
            out[b, nq * 1024 + rank * BAND : nq * 1024 + (rank + 1) * BAND] = op[
                nq * BAND : (nq + 1) * BAND
            ]
    return out, attn
